# revision 58
# baseline (speedup 1.0000x reference)
"""Trainium2 Bass kernel for a dense attention layer.

Reference computation (B=4, Q=K=4096, IN=D=256):
    q = queries @ Wq.T + bq ; k = keys @ Wk.T + bk ; v = values @ Wv.T + bv
    scores = (q @ k.T  masked to key < mask[q] with -1e9) / sqrt(D)
    out = softmax(scores) @ v @ Wo.T + bo

Math restructuring (weight-only algebra + bias handling precomputed on
host; every GEMM of the reference runs on device, fused):
    scores  == queries @ A @ keys.T + s_k (+ per-query terms that cancel in
               softmax), A = Wq.T @ Wk, s_k = keys_k . (Wk.T bq).
    out     == (P' @ [w*values]) -> normalize by (P' @ w) -> @ C + bo', with
               C = Wv.T @ Wo.T (the fused V/out projection, applied on
               device in the epilogue), w_k = exp(s_k/16) (host-computed
               bias weight folded into the values and the denominator),
               bo' = Wo @ bv + bo.
    The attention weighted sum uses fp8 DoubleRow matmuls with an exact
    host-side fp8 residual decomposition of w*values (v8 + v2), so PV
    costs 2 x 0.5 cycles/row at ~bf16 accuracy on the V side.  Probs are
    fp8 (the dominant remaining error term ~1e-2 of the 2e-2 budget).

Structure per core (data-parallel over B x 2, queries sorted by mask
length and dealt round-robin; chunk trip counts and column ranges baked
from the actual mask values, shared SPMD graph uses min/max over cores):
  - 256-key chunks: 2 fp8 DR score matmuls (16*A scale, exp scale 1/256)
    -> ONE merged exp per chunk on ACT (the roofline: ~sum(mask)/128
    elements) -> boundary mask multiply (DVE, host-built {0,1} tiles) ->
    per query-subtile 2 fp8 DR PV matmuls (v8 + residual) + a 1-column
    DR denominator matmul.
  - PSUM: 2x [128,2,512] score ring, 2 banks of paired att accumulators
    (single zeroing start per bank, per-column stops, skip_group_check),
    1 denominator bank, 1 epilogue bank.
  - Epilogue per subtile (emitted as soon as its accumulators close):
    att->SBUF copy, PE transpose, C-projection (bf16), out = po/denom
    + bo' (DVE scalar_tensor_tensor), per-subtile output DMA.
  - All engines balanced: PE matmuls ~28us, ACT exp ~37us (bottleneck),
    DVE casts/masks/epilogue ~28us, SP DMAs; deep pb ring (12) and a
    deferred-PV pending queue keep the exp stream dense.
"""

import numpy as np
import ml_dtypes

import concourse.bass as bass
import concourse.mybir as mybir
from concourse import bacc
from concourse.tile import TileContext
from concourse.masks import make_identity
from concourse.bass_utils import run_bass_kernel_spmd

BF16 = ml_dtypes.bfloat16
FP8 = ml_dtypes.float8_e4m3

B, Q, KLEN, IN, D = 4, 4096, 4096, 256, 256
N_CORES = 8
QS = Q // 2            # queries per core
TQ = 512               # query tile
NQT = QS // TQ         # query tiles per core
KC = 256               # key chunk (DoubleRow contraction)
NKC = KLEN // KC       # 16

F32 = mybir.dt.float32
F8 = mybir.dt.float8e4
BF = mybir.dt.bfloat16

# DMA piece boundaries
KP = [0, 512, 1024, 2048, 3072, 4096]   # keysT columns
QP = [0, 512, 1024, 2048]               # queriesT columns
VP = [0, 4, 8, 12, 16]                  # v8/v2 chunk-index pieces


def _make_plan(sorted_masks):
    """sorted_masks: [N_CORES, QS] ascending per-core mask lengths."""
    n_chunks, zqs, zxs, ecs = [], [], [], []
    for t in range(NQT):
        seg = sorted_masks[:, t * TQ:(t + 1) * TQ]
        nc_t = int(np.ceil(seg.max() / KC))
        zq_t, zx_t, ec_t = [], [], []
        for j in range(nc_t):
            zx = int(min(np.searchsorted(seg[c], KC * j, side="right")
                         for c in range(N_CORES)))
            e0 = int(max(np.searchsorted(seg[c], KC * j + 128, side="left")
                         for c in range(N_CORES)))
            e1 = int(max(np.searchsorted(seg[c], KC * j + 256, side="left")
                         for c in range(N_CORES)))
            zq_t.append((zx // 128) * 128)
            zx_t.append(zx)
            ec_t.append((max(e0, zx), max(e1, zx)))
        n_chunks.append(nc_t)
        zqs.append(zq_t)
        zxs.append(zx_t)
        ecs.append(ec_t)
    return n_chunks, zqs, zxs, ecs


def _vd_slices(plan):
    """(t, j, c, zx, e_c, offset) entries of the concatenated validity
    tensor + per-tile column ranges."""
    n_chunks, zqs, zxs, ecs = plan
    entries, off = [], 0
    tile_ranges = []
    for t in range(NQT):
        t0 = off
        for j in range(n_chunks[t]):
            zx = zxs[t][j]
            for c in range(2):
                e = ecs[t][j][c]
                if e > zx:
                    entries.append((t, j, c, zx, e, off))
                    off += e - zx
        tile_ranges.append((t0, off))
    return entries, max(off, 1), tile_ranges


def _bcast_ap(handle, parts, free):
    ap = handle.ap()
    return bass.AP(tensor=ap.tensor, offset=ap.offset, ap=[[0, parts], [1, free]])


def build_bass(plan, pipe=3):
    n_chunks, zqs, zxs, ecs = plan
    vd_entries, nvd, vd_tiles = _vd_slices(plan)
    nc = bacc.Bacc(
        "TRN2",
        target_bir_lowering=False,
        debug=False,
        enable_asserts=False,
        num_devices=1,
    )

    qT_d = nc.declare_dram_parameter("qT", [2, 128, QS], F8, isOutput=False)
    kT_d = nc.declare_dram_parameter("kT", [2, 128, KLEN], F8, isOutput=False)
    v8_d = nc.declare_dram_parameter("v8", [128, NKC, 2, D], F8, isOutput=False)
    v2_d = nc.declare_dram_parameter("v2r", [128, NKC, 2, D], F8, isOutput=False)
    wc_d = nc.declare_dram_parameter("wc8", [128, NKC, 2, 1], F8, isOutput=False)
    A_d = nc.declare_dram_parameter("Amat", [128, 2, 256], F8, isOutput=False)
    C_d = nc.declare_dram_parameter("Cmat", [128, 2, 256], BF, isOutput=False)
    bo_d = nc.declare_dram_parameter("bop", [1, D], F32, isOutput=False)
    vd_d = nc.declare_dram_parameter("vdcat", [128, nvd], F8, isOutput=False)
    out_d = nc.declare_dram_parameter("out", [QS, D], BF, isOutput=True)

    with TileContext(nc) as tc:
        with (
            tc.tile_pool(name="consts", bufs=1) as consts,
            tc.tile_pool(name="probs", bufs=12) as probs,
            tc.tile_pool(name="recp", bufs=3) as recp,
            tc.tile_pool(name="attsb", bufs=2) as attsbp,
            tc.tile_pool(name="attTsb", bufs=4) as attTsbp,
            tc.tile_pool(name="outsb", bufs=2) as outsb,
            tc.tile_pool(name="scps", bufs=2, space="PSUM") as scps,
            tc.tile_pool(name="attps", bufs=1, space="PSUM") as attps,
            tc.tile_pool(name="dnps", bufs=1, space="PSUM") as dnps,
            tc.tile_pool(name="epps", bufs=1, space="PSUM") as epps,
        ):
            # ---- SBUF constants / staged inputs ---------------------------
            A_s = consts.tile([128, 2, 256], F8, tag="A")
            C_s = consts.tile([128, 2, 256], BF, tag="C")
            bo_s = consts.tile([128, D], F32, tag="bo")
            vdc = consts.tile([128, nvd], F8, tag="vdc")
            gT_s = consts.tile([128, 2, QS], F8, tag="gT")
            v8_s = consts.tile([128, NKC, 2, D], F8, tag="v8")
            v2_s = consts.tile([128, NKC, 2, D], F8, tag="v2")
            wc_s = consts.tile([128, NKC, 2, 1], F8, tag="wc")
            ident = consts.tile([128, 128], BF, tag="ident")
            make_identity(nc, ident)

            kTt = [consts.tile([128, 2, KP[i + 1] - KP[i]], F8, tag=f"kT{i}",
                               name=f"kT{i}") for i in range(len(KP) - 1)]
            qTt = [consts.tile([128, 2, QP[i + 1] - QP[i]], F8, tag=f"qT{i}",
                               name=f"qT{i}") for i in range(len(QP) - 1)]

            def dma_piece(dram, tiles, bounds, i):
                nc.sync.dma_start(
                    out=tiles[i][:, :, :],
                    in_=dram[:, :, bounds[i]:bounds[i + 1]].rearrange(
                        "c p x -> p c x"))

            def v_piece(dram, tile, i):
                nc.sync.dma_start(out=tile[:, VP[i]:VP[i + 1], :, :],
                                  in_=dram[:, VP[i]:VP[i + 1], :, :])

            def vd_piece(t):
                lo, hi = vd_tiles[t]
                if hi > lo:
                    nc.sync.dma_start(out=vdc[:, lo:hi], in_=vd_d[:, lo:hi])

            # DMA order tuned so consumers find data landed (single SP queue)
            dma_piece(kT_d, kTt, KP, 0)
            nc.scalar.dma_start(out=A_s[:, :, :], in_=A_d.ap())
            dma_piece(qT_d, qTt, QP, 0)
            dma_piece(kT_d, kTt, KP, 1)
            nc.scalar.dma_start(out=wc_s[:, :, :, :], in_=wc_d.ap())
            nc.scalar.dma_start(
                out=qTt[1][:, :, :],
                in_=qT_d[:, :, QP[1]:QP[2]].rearrange("c p x -> p c x"))
            vd_piece(0)
            v_piece(v8_d, v8_s, 0)
            v_piece(v2_d, v2_s, 0)
            nc.sync.dma_start(out=C_s[:, :, :], in_=C_d.ap())
            nc.sync.dma_start(out=bo_s[:, :], in_=_bcast_ap(bo_d, 128, D))
            vd_piece(1)
            v_piece(v8_d, v8_s, 1)
            v_piece(v2_d, v2_s, 1)
            dma_piece(kT_d, kTt, KP, 2)
            dma_piece(qT_d, qTt, QP, 2)
            vd_piece(2)
            v_piece(v8_d, v8_s, 2)
            v_piece(v2_d, v2_s, 2)
            dma_piece(kT_d, kTt, KP, 3)
            v_piece(v8_d, v8_s, 3)
            v_piece(v2_d, v2_s, 3)
            dma_piece(kT_d, kTt, KP, 4)
            vd_piece(3)

            import bisect

            def kslice(lo, hi):
                g = bisect.bisect_right(KP, lo) - 1
                assert hi <= KP[g + 1], (lo, hi)
                return kTt[g][:, :, lo - KP[g]:hi - KP[g]]

            def qslice2(lo, hi):
                g = bisect.bisect_right(QP, lo) - 1
                assert hi <= QP[g + 1], (lo, hi)
                return qTt[g][:, :, lo - QP[g]:hi - QP[g]]

            # gT half-group (t, h): one DR matmul + fp8 cast.  Tile 0's
            # h=1 cast runs on the (still idle) ACT engine to cut the head.
            def make_ggroup(t, h):
                def emit():
                    q0 = t * TQ
                    ps = scps.tile([128, 2, TQ], F32, tag="sc")
                    nc.tensor.matmul(
                        ps[:, h, :],
                        A_s[:, :, h * 128:(h + 1) * 128],
                        qslice2(q0, q0 + TQ),
                        start=True, stop=True,
                        perf_mode=mybir.MatmulPerfMode.DoubleRow)
                    if t == 0 and h == 1:
                        nc.scalar.copy(gT_s[:, h, q0:q0 + TQ], ps[:, h, :])
                    else:
                        nc.vector.tensor_copy(out=gT_s[:, h, q0:q0 + TQ],
                                              in_=ps[:, h, :])
                return emit

            filler_slots = {1: [make_ggroup(1, 0)], 2: [make_ggroup(1, 1)],
                            6: [make_ggroup(2, 0)], 7: [make_ggroup(2, 1)],
                            11: [make_ggroup(3, 0)], 12: [make_ggroup(3, 1)]}

            # ---- prologue -------------------------------------------------
            wu = epps.tile([128, 512], F32, tag="ep")
            wub = wu[:, 0:64].bitcast(BF)
            nc.tensor.transpose(wub[:, 0:128], ident[:, :], ident[:, :])
            make_ggroup(0, 0)()
            make_ggroup(0, 1)()

            vd_index = {(t, j, c): (zx, e, off)
                        for (t, j, c, zx, e, off) in vd_entries}

            # ---- attention ------------------------------------------------
            NS = TQ // 128
            ep_queue = []
            gchunk = [0]
            pending = []   # (issue_fn, j, pb, zq, ep_maker_or_None)

            def pop_pending():
                fn, j, pb, zq, eps = pending.pop(0)
                fn(j, pb, zq)
                ep_queue.extend(eps)

            def make_tile_ep(t, attt, atts, dn, is_last):
                """Per-tile epilogue state.  cA/cB eager per subtile (free
                the att banks, transpose + C-proj); cC late (rec + STT +
                out DMA, gated on the dn bank group closing)."""
                q0 = t * TQ
                rec = recp.tile([128, NS], F32, tag="rec")
                att_sb = attsbp.tile([128, NS, D], BF, tag="attsb")
                ot = outsb.tile([128, NS, D], BF, tag="ot")
                pos = []

                def bank_copy(b):
                    def cA(b=b):
                        if is_last:
                            nc.scalar.copy(att_sb[:, 2 * b:2 * b + 2, :],
                                           attt[b][:, :, :])
                        else:
                            nc.vector.tensor_copy(
                                out=att_sb[:, 2 * b:2 * b + 2, :],
                                in_=attt[b][:, :, :])
                    return cA

                def sub_AB(s):
                    def cB(s=s):
                        if s % 2 == 0 or not is_last:
                            ep = epps.tile([128, 512], F32, tag="ep")
                        else:
                            ept = scps.tile([128, 2, TQ], F32, tag="sc",
                                            name=f"ep{s}")
                            ep = ept[:, 0, :]
                        tpb = ep[:, 0:128].bitcast(BF)   # [128, 256] bf16
                        for c in range(2):
                            nc.tensor.transpose(
                                tpb[:, c * 128:(c + 1) * 128],
                                att_sb[:, s, c * 128:(c + 1) * 128],
                                ident[:, :])
                        attT_sb = attTsbp.tile([128, 2, 128], BF, tag="attTsb")
                        nc.vector.tensor_copy(out=attT_sb[:, :, :],
                                              in_=tpb[:, :])
                        for c in range(2):
                            nc.tensor.matmul(ep[:, 128:128 + D],
                                             attT_sb[:, c, :],
                                             C_s[:, c, :],
                                             start=(c == 0), stop=(c == 1))
                        pos.append(ep)
                    return [cB]

                def sub_C(s, last):
                    def cC(s=s, last=last):
                        nc.vector.reciprocal(rec[:, s:s + 1], dn[:, s:s + 1])
                        nc.vector.scalar_tensor_tensor(
                            ot[:, s, :], pos[s][:, 128:128 + D],
                            rec[:, s:s + 1], bo_s[:, :],
                            mybir.AluOpType.mult, mybir.AluOpType.add)
                        out_slice = out_d[q0 + 128 * s:q0 + 128 * (s + 1),
                                          :].rearrange("(o p) d -> p o d",
                                                       p=128)
                        nc.sync.dma_start(out=out_slice, in_=ot[:, s:s + 1, :])
                    return cC
                return sub_AB, sub_C, bank_copy

            for t in range(NQT):
                nch = n_chunks[t]
                q0 = t * TQ
                attt = [attps.tile([128, 2, D], F32, tag=f"attb{i}",
                                   name=f"attb{i}") for i in range(2)]
                atts = [attt[s // 2][:, s % 2, :] for s in range(NS)]
                dn = dnps.tile([128, NS], F32, tag="dn")
                last_j = [max(j for j in range(nch) if zqs[t][j] < (s + 1) * 128)
                          for s in range(NS)]

                sub_AB, sub_C, bank_copy = make_tile_ep(
                    t, attt, atts, dn, t == NQT - 1)

                def issue_pv(j, pb, zq, atts=atts, dn=dn, last_j=last_j,
                             nch=nch):
                    for s in range(zq // 128, NS):
                        # one bank-zeroing start per shared bank; per-column
                        # stop on that column's last write (sim bookkeeping)
                        nc.tensor.matmul(
                            atts[s],
                            pb[:, :, s * 128:(s + 1) * 128],
                            v8_s[:, j, :, :],
                            start=(j == 0 and s % 2 == 0), stop=False,
                            perf_mode=mybir.MatmulPerfMode.DoubleRow,
                            skip_group_check=True)
                        nc.tensor.matmul(
                            atts[s],
                            pb[:, :, s * 128:(s + 1) * 128],
                            v2_s[:, j, :, :],
                            start=False, stop=(j == last_j[s]),
                            perf_mode=mybir.MatmulPerfMode.DoubleRow,
                            skip_group_check=True)
                        nc.tensor.matmul(
                            dn[:, s:s + 1],
                            pb[:, :, s * 128:(s + 1) * 128],
                            wc_s[:, j, :, :],
                            start=(j == 0 and s == 0),
                            stop=(j == last_j[s]),
                            perf_mode=mybir.MatmulPerfMode.DoubleRow,
                            skip_group_check=True)

                for j in range(nch):
                    zq, zx = zqs[t][j], zxs[t][j]
                    sc = scps.tile([128, 2, TQ], F32, tag="sc")
                    for c in range(2):
                        nc.tensor.matmul(
                            sc[:, c, zx:],
                            kslice(KC * j + 128 * c, KC * j + 128 * (c + 1)),
                            gT_s[:, :, q0 + zx:q0 + TQ],
                            start=True, stop=True,
                            perf_mode=mybir.MatmulPerfMode.DoubleRow)
                    pb = probs.tile([128, 2, TQ], F8, tag="pb")
                    nc.scalar.activation(pb[:, :, zx:], sc[:, :, zx:],
                                         mybir.ActivationFunctionType.Exp,
                                         scale=1.0 / 256.0)
                    if zx > zq:
                        nc.vector.memset(pb[:, :, zq:zx], 0.0)
                    for c in range(2):
                        ent = vd_index.get((t, j, c))
                        if ent is not None:
                            vzx, ve, off = ent
                            nc.vector.tensor_mul(
                                pb[:, c, vzx:ve], pb[:, c, vzx:ve],
                                vdc[:, off:off + (ve - vzx)])
                    for f in filler_slots.pop(gchunk[0], ()):
                        f()
                    gchunk[0] += 1
                    if ep_queue:
                        ep_queue.pop(0)()
                    eps = []
                    for b in range(2):
                        if last_j[2 * b + 1] == j:
                            eps.append(bank_copy(b))
                            eps.extend(sub_AB(2 * b))
                            eps.append(sub_C(2 * b, False))
                            eps.extend(sub_AB(2 * b + 1))
                            eps.append(sub_C(2 * b + 1, 2 * b + 1 == NS - 1))
                    pending.append((issue_pv, j, pb, zq, eps))
                    peff = 1 if t == NQT - 1 and j >= nch - 6 else pipe
                    while len(pending) > peff:
                        pop_pending()
            while pending:
                pop_pending()
                if ep_queue:
                    ep_queue.pop(0)()
            for i in sorted(filler_slots):
                for f in filler_slots.pop(i):
                    f()
            while ep_queue:
                ep_queue.pop(0)()

    nc.compile()
    return nc


def prepare(inputs):
    """Host-side prep: weight algebra, sharding, packing, validity tiles."""
    queries = np.asarray(inputs["queries"], np.float32)
    keys = np.asarray(inputs["keys"], np.float32)
    values = np.asarray(inputs["values"], np.float32)
    mask = np.asarray(inputs["mask"])
    w = {k: np.asarray(inputs[k], np.float32)
         for k in ("Wq", "bq", "Wk", "bk", "Wv", "bv", "Wo", "bo")}

    A = w["Wq"].T @ w["Wk"]                    # [in, in]
    C = w["Wv"].T @ w["Wo"].T                  # [in, D]
    u = w["Wk"].T @ w["bq"]                    # [in]
    bop = w["Wo"] @ w["bv"] + w["bo"]          # [D]

    def packA(M, dt):  # [256, X] -> [128, 2, X] with d=(c*128+p)
        return np.ascontiguousarray(
            M.reshape(2, 128, M.shape[1]).transpose(1, 0, 2)).astype(dt)

    shared = {
        "Amat": packA(16.0 * A, FP8),
        "Cmat": packA(C, BF16),
        "bop": bop.reshape(1, D).astype(np.float32),
    }

    in_maps, perms = [], []
    sorted_masks = np.zeros((N_CORES, QS), np.int64)
    for b in range(B):
        order = np.argsort(mask[b], kind="stable")
        keysT = np.ascontiguousarray(keys[b].T).reshape(2, 128, KLEN).astype(FP8)
        wvec = np.exp(keys[b] @ u / 16.0)          # [K] per-key softmax weight
        vaug = values[b] * wvec[:, None]           # [K, 256]
        v8 = vaug.astype(FP8)
        v2 = (vaug - v8.astype(np.float64)).astype(FP8)

        def packV(M):  # [K, 256] -> [128, NKC, 2, 256], key = 256j+128c+p
            return np.ascontiguousarray(
                M.reshape(NKC, 2, 128, D).transpose(2, 0, 1, 3))
        v8p, v2p = packV(v8), packV(v2)
        wc8 = np.ascontiguousarray(
            wvec.astype(FP8).reshape(NKC, 2, 128, 1).transpose(2, 0, 1, 3))
        for h in range(2):
            c = 2 * b + h
            idx = order[h::2]
            perms.append(idx)
            sorted_masks[c] = mask[b][idx]
            qT = np.ascontiguousarray(queries[b][idx].T)
            in_maps.append({
                "qT": qT.reshape(2, 128, QS).astype(FP8),
                "kT": keysT,
                "v8": v8p,
                "v2r": v2p,
                "wc8": wc8,
                **shared,
            })
    plan = _make_plan(sorted_masks)

    vd_entries, nvd, _vdt = _vd_slices(plan)
    key_idx = np.arange(128)
    for c in range(N_CORES):
        vd = np.zeros((128, nvd), FP8)
        sm = sorted_masks[c]
        for (t, j, ch, zx, e, off) in vd_entries:
            m = sm[t * TQ + zx:t * TQ + e]                  # [e-zx]
            kv = KC * j + 128 * ch + key_idx                # [128]
            vd[:, off:off + (e - zx)] = (m[None, :] > kv[:, None]).astype(FP8)
        in_maps[c]["vdcat"] = vd
    return in_maps, plan, perms


def assemble(results, perms):
    out = np.zeros((B, Q, D), np.float32)
    for c in range(N_CORES):
        out[c // 2][perms[c]] = np.asarray(results[c]["out"], np.float32)
    return out


def kernel(**inputs) -> np.ndarray:
    in_maps, plan, perms = prepare(inputs)
    nc = build_bass(plan)
    res = run_bass_kernel_spmd(nc, in_maps, core_ids=list(range(N_CORES)))
    return assemble(res.results, perms)
